# revision 4
# baseline (speedup 1.0000x reference)
"""Embedding lookup kernel for TRN2 (8 NeuronCores, SPMD data-parallel).

out[0, t, :] = W[:, idx[t]] + b   for t in [0, 32*8192)

Strategy: host precomputes table = W.T + b  ([100000, 128] f32, 512B rows)
and replicates it to all 8 cores; tokens are sharded 32768/core. Each core
gathers its rows via SWDGE indirect DMA (128 rows per instruction — the HW
consumes one offset per dest partition), then streams tiles back to HBM.
"""

import numpy as np

import concourse.bacc as bacc
import concourse.mybir as mybir
import concourse.tile as tile
from concourse import bass
from concourse.bass_utils import run_bass_kernel_spmd

NCORES = 8
B, S = 32, 8192
TOKENS = B * S              # 262144
T = TOKENS // NCORES        # 32768 tokens per core
V = 100000
D = 128                     # embedding dim; 512 bytes per row (f32)
G = 8                       # gathers (128 rows each) per store group
NGRP = T // (128 * G)       # 32 store groups per core
NGATH = T // 128            # 256 gather instructions per core

_compiled_nc = None


def _build():
    nc = bacc.Bacc("TRN2", target_bir_lowering=False, debug=False)
    # idx layout: [128, NGATH] int32, column g holds tokens [128g, 128(g+1))
    # one per partition (host pre-transposes).
    idx_d = nc.dram_tensor("idx", [128, NGATH], mybir.dt.int32,
                           kind="ExternalInput").ap()
    tab_d = nc.dram_tensor("tab", [V, D], mybir.dt.float32,
                           kind="ExternalInput").ap()
    out_d = nc.dram_tensor("out", [T, D], mybir.dt.float32,
                           kind="ExternalOutput").ap()

    with tile.TileContext(nc) as tc:
        with tc.tile_pool(name="data", bufs=3) as dp, \
             tc.tile_pool(name="idxp", bufs=1) as ip:
            it = ip.tile([128, NGATH], mybir.dt.int32)
            nc.sync.dma_start(out=it[:], in_=idx_d[:])
            for c in range(NGRP):
                dt_ = dp.tile([128, G * D], mybir.dt.float32)
                for g in range(G):
                    nc.gpsimd.indirect_dma_start(
                        out=dt_[:, g * D:(g + 1) * D],
                        out_offset=None,
                        in_=tab_d[:],
                        in_offset=bass.IndirectOffsetOnAxis(
                            ap=it[:, c * G + g:c * G + g + 1], axis=0),
                    )
                # store group: SBUF (p, g*D+d) -> DRAM row c*G*128 + g*128 + p
                dst = out_d[c * G * 128:(c + 1) * G * 128, :] \
                    .rearrange("(g p) d -> p g d", p=128)
                nc.sync.dma_start(
                    out=dst, in_=dt_[:].rearrange("p (g d) -> p g d", g=G))
    nc.compile()
    return nc


def _get_nc():
    global _compiled_nc
    if _compiled_nc is None:
        _compiled_nc = _build()
    return _compiled_nc


def _make_in_maps(X, W, b):
    X = np.asarray(X)
    W = np.asarray(W, dtype=np.float32)
    b = np.asarray(b, dtype=np.float32)

    idx = np.ascontiguousarray(X.reshape(-1).astype(np.int32))
    table = np.ascontiguousarray(W.T) + b[None, :]

    return [
        {
            # [T] -> [NGATH, 128] -> transpose -> [128, NGATH]
            "idx": np.ascontiguousarray(
                idx[c * T:(c + 1) * T].reshape(NGATH, 128).T),
            "tab": table,
        }
        for c in range(NCORES)
    ]


def _gather_out(res):
    out = np.concatenate(
        [res.results[c]["out"] for c in range(NCORES)], axis=0
    )
    return out.reshape(1, TOKENS, D)


def kernel(X, W, b):
    in_maps = _make_in_maps(X, W, b)
    res = run_bass_kernel_spmd(_get_nc(), in_maps, list(range(NCORES)))
    return _gather_out(res)
